# revision 87
# baseline (speedup 1.0000x reference)
"""Trainium2 Bass kernel: autoregressive wavefunction log-prob (N=64, B=2048, H=512).

Sharding: step axis N across 8 cores, round-robin (core c owns global steps
c, c+8, ..., c+56).  Per local step i:
    h1 = relu(c_i + D_i^T sigma)        sigma-encoding of the one-hot prefix
    h2 = relu(h1 @ W2_i + b2_i)
    d  = h2 @ (W3_i[:,0]-W3_i[:,1]) + (b3_i[0]-b3_i[1])
    -logp_i = softplus(-sigma_i * d)

sigma-encoding: W1[2j+s] = M_j + sigma*D_j with sigma = s0-s1 in {+1,-1},
so h1_i = c_i + D_i^T sigma with contraction K=64.  Steps 2p/2p+1 run as
row-tiled tiles (0,0)/(64,0); a post-scheduling pass (LOCKSTEP) re-pairs
their matmuls adjacently so the two tiles stream concurrently (~2 cols/
cycle).  Layer 2 runs fp8 DoubleRow (K=256/pass) at the 1-col/cycle
roofline.  Layer 3 runs fp8 DoubleRow over h2' (pairs of k-chunks packed
in the moving layout), 32-row outputs at column position 0 (the only legal
DoubleRow position): batch chunk b lands in PSUM bank b//2 at rows
8*(b%2)+j via per-chunk weight diagonals.

Slot 0 (global step c) depends only on the <=2^c <= 128 distinct
sigma-prefix patterns: layers 1-3 run over an enumerated 128-pattern chunk
(exact), the tail emits both sigma=+1/-1 branches per pattern (bank-0 rows
0/8), and the host gathers per-sample values -- ~14% of the full-B work.

Scale folding keeps casts exact: D,c x8 (h1' = 8*h1 fp8), W2 x8 fp8,
b2 x64, h2' = 64*h2 fp8 (max ~158 < 240), W3D /64.  Tail:
-logp = ln(exp(u) + 1) with the +1 folded into the Ln activation bias,
bank-0 chunks flushed first so its tail overlaps bank 1's matmuls.
"""

import numpy as np
import ml_dtypes

import concourse.bass as bass
import concourse.mybir as mybir
import concourse.tile as tile
from concourse.bass_utils import run_bass_kernel_spmd

N, B, H = 64, 2048, 512
NCORES = 8
NSTEP = N // NCORES          # 8 local steps per core
NPAIR = NSTEP // 2           # 4 row-tiled step pairs
BCH = 512                    # batch chunk (one fp32 PSUM bank)
NB = B // BCH                # 4
NM = H // 128                # 4 h-chunks

BF = mybir.dt.bfloat16
F32 = mybir.dt.float32
FP8 = mybir.dt.float8e4
NPBF = ml_dtypes.bfloat16
NPF8 = ml_dtypes.float8_e4m3

SH1 = 8.0      # h1 scale (folded into D, c)
SW2 = 8.0      # W2 fp8 scale (8 keeps h2' = 64*h2 under fp8e4 max 240)
SZ2 = SH1 * SW2  # 64; folded into b2 and out of W3D

# CF32 packed const columns
C_C1 = 0                       # [0, 32)   c' bias, col j*NM+m
C_B2 = 32                      # [32, 64)  64*b2, col j*NM+m
C_B3 = 64                      # [64, 65)  b3d (rows j and 8+j)
C_NS = 65                      # [65, 65+2*BCH) -sigma, bank-major: bank t
#   holds batch chunks 2t (rows 0-7) and 2t+1 (rows 8-15)
NCF = C_NS + 2 * BCH

TRACE = False
LAST_RESULT = None
LOCKSTEP = True    # pair row-tiled L1 matmuls in the PE stream


def _thin_sem_incs(nc):
    """Drop PE-semaphore increments whose cumulative value nobody waits on
    (each then_inc serializes ~26ns at the EVT_SEM register); renumber the
    surviving waits.  PE executes in order, so the v-th increment firing
    implies all prior PE work retired.  Only touches semaphores updated
    exclusively by PE instructions with update_value 1."""
    blocks = [blk for fn in nc.m.functions for blk in fn.blocks]
    upd = {}
    waited = {}
    ok = {}
    for blk in blocks:
        for inst in blk.instructions:
            si = inst.sync_info
            if si is None:
                continue
            for w in (si.on_wait or []):
                if w.wait_mode == "sem-ge-imm" and w.wait_value is not None:
                    waited.setdefault(w.id, set()).add(w.wait_value)
                else:
                    ok[w.id] = False
            for u in (si.on_update or []):
                lst = upd.setdefault(u.id, [])
                lst.append((inst, (lst[-1][1] if lst else 0) + (u.update_value or 0)))
                is_pe = (
                    inst.engine == mybir.EngineType.PE
                    and u.update_value == 1
                    and getattr(u, "update_mode", "sem-inc") == "sem-inc"
                )
                if not is_pe:
                    ok[u.id] = False
    for sem_id, updates in upd.items():
        if not ok.get(sem_id, True):
            continue
        keep_vals = sorted(v for v in waited.get(sem_id, set()) if v > 0)
        remap = {}
        new_cum = 0
        ki = 0
        for inst, cum in updates:
            if ki < len(keep_vals) and cum >= keep_vals[ki]:
                while ki < len(keep_vals) and keep_vals[ki] <= cum:
                    remap[keep_vals[ki]] = new_cum + 1
                    ki += 1
                new_cum += 1
            else:
                si = inst.sync_info
                nu = [u for u in (si.on_update or []) if u.id != sem_id]
                inst.sync_info = mybir.SyncInfo(
                    on_wait=list(si.on_wait or []), on_update=nu
                )
        for blk in blocks:
            for inst in blk.instructions:
                si = inst.sync_info
                if si is None or not si.on_wait:
                    continue
                changed = False
                nw = []
                for w in si.on_wait:
                    if w.id == sem_id and w.wait_value and w.wait_value > 0:
                        nw.append(mybir.SyncWait(
                            sync_type=w.sync_type, id=w.id,
                            ant_name=w.ant_name, wait_mode=w.wait_mode,
                            wait_value=remap[w.wait_value],
                        ))
                        changed = True
                    else:
                        nw.append(w)
                if changed:
                    inst.sync_info = mybir.SyncInfo(
                        on_wait=nw, on_update=list(si.on_update or [])
                    )
    return nc


def _elide_redundant_ldweights(nc):
    """Tile lowers every matmul into an Ldweights+Matmult pair.  Consecutive
    matmuls sharing the stationary operand in the SAME array tile don't need
    the repeated loads (each tile_position owns independent weight
    registers, untouched by Matmults or by loads into other tiles).  A
    dropped load's semaphore waits are kept as an EventSemaphore in the PE
    stream."""
    for fn in nc.m.functions:
        for blk in fn.blocks:
            new = []
            last_key = {}  # tile_position -> weights key
            def region(inst):
                pos = inst.tile_position or (0, 0)
                ts = inst.tile_size or (128, 128)
                return (pos[0], pos[0] + ts[0], pos[1], pos[1] + ts[1])

            for inst in blk.instructions:
                if isinstance(inst, mybir.InstMatmult):
                    if inst.is_transpose:
                        last_key = {}  # transpose streams data in as weights
                    new.append(inst)
                    continue
                if isinstance(inst, mybir.InstLdweights):
                    a = inst.ins[0]
                    r = region(inst)
                    pos = (r[0], r[2])
                    key = (
                        a.memref, a.offset, str(a.ap), str(a.dtype),
                        str(inst.perf_mode), str(inst.is_transpose), r,
                    )
                    if last_key.get(pos) == key:
                        si = inst.sync_info
                        if si is not None and (si.on_wait or si.on_update):
                            new.append(mybir.InstEventSemaphore(
                                name=f"{inst.name}-ldwelide",
                                engine=inst.engine,
                                sync_info=si,
                            ))
                        continue
                    # this load clobbers every overlapping array region
                    for pos2 in list(last_key):
                        k2 = last_key[pos2]
                        r2 = k2[6]
                        if r[0] < r2[1] and r2[0] < r[1] and r[2] < r2[3] \
                                and r2[2] < r[3]:
                            del last_key[pos2]
                    last_key[pos] = key
                    new.append(inst)
                    continue
                # other PE-stream instructions (EventSemaphore, Drain,
                # RegisterMove, branches) don't touch the weight registers
                new.append(inst)
            blk.instructions = new
    return nc


def _drop_pe_self_waits(nc):
    """Remove PE-instruction waits on semaphores incremented exclusively by
    PE instructions.  The PE executes and drains strictly in program order,
    so any PE->PE ordering a semaphore enforces already holds — but the
    wait itself blocks ISSUE, which serializes row/column-tiled matmuls
    that the array could run concurrently.  Cross-engine waits (ACT/DVE/DMA
    waiting on PE, or PE waiting on them) are untouched."""
    blocks = [blk for fn in nc.m.functions for blk in fn.blocks]
    pe_only = {}
    for blk in blocks:
        for inst in blk.instructions:
            si = inst.sync_info
            if si is None:
                continue
            for u in (si.on_update or []):
                if inst.engine != mybir.EngineType.PE:
                    pe_only[u.id] = False
                elif u.id not in pe_only:
                    pe_only[u.id] = True
    for blk in blocks:
        new = []
        for inst in blk.instructions:
            si = inst.sync_info
            if inst.engine == mybir.EngineType.PE and si is not None and si.on_wait:
                kept = [
                    w for w in si.on_wait
                    if not (w.wait_mode == "sem-ge-imm" and pe_only.get(w.id, False))
                ]
                if len(kept) != len(si.on_wait):
                    if (not kept and not (si.on_update or [])
                            and isinstance(inst, mybir.InstEventSemaphore)):
                        continue  # nothing left to do
                    inst.sync_info = mybir.SyncInfo(
                        on_wait=kept, on_update=list(si.on_update or [])
                    )
            new.append(inst)
        blk.instructions = new
    return nc


def _pair_l1_lockstep(nc):
    """Interleave the row-tiled layer-1 matmul pairs in the final PE stream.

    Two concurrent 64-row tiles stream their moving columns in lockstep off
    the shared 128-partition XBUS read (~2 cols/cycle aggregate), but only
    when the (64,0)-tile matmul issues immediately after its (0,0) partner.
    The Tile scheduler's cost model serializes tiles, so it often groups
    same-tile matmuls instead.  This pass moves each hi-tile Ldweights/
    Matmult to sit right behind its lo partner and renumbers wait values on
    PE-exclusive semaphores so every wait still triggers on the same
    instruction.  Must run AFTER _drop_pe_self_waits (no PE->PE waits may
    remain) and BEFORE _elide_redundant_ldweights/_thin_sem_incs."""
    PE = mybir.EngineType.PE

    def l1_half(inst):
        if not isinstance(inst, (mybir.InstMatmult, mybir.InstLdweights)):
            return None
        if getattr(inst, "is_transpose", False):
            return None
        if inst.tile_size != (64, 128) or inst.tile_position is None:
            return None
        return 0 if inst.tile_position[0] == 0 else 1

    for fn in nc.m.functions:
        for blk in fn.blocks:
            insts = blk.instructions
            pe_pos = [i for i, ins in enumerate(insts)
                      if ins.engine == PE]
            pe = [insts[i] for i in pe_pos]
            if not pe:
                continue

            # semaphores updated exclusively by PE instructions in this blk
            pe_only = {}
            for ins in insts:
                si = ins.sync_info
                if si is None:
                    continue
                for u in (si.on_update or []):
                    if ins.engine != PE:
                        pe_only[u.id] = False
                    elif u.id not in pe_only:
                        pe_only[u.id] = True
            safe_sems = {s for s, ok in pe_only.items() if ok}

            def movable(ins):
                si = ins.sync_info
                if si is None:
                    return True
                return all(u.id in safe_sems for u in (si.on_update or []))

            # match each hi instruction to its lo partner
            lo_mm, lo_ldw = {}, {}
            for k, ins in enumerate(pe):
                if l1_half(ins) == 0:
                    if isinstance(ins, mybir.InstMatmult):
                        a = ins.ins[0]
                        lo_mm.setdefault(
                            (a.memref, a.offset, ins.ins[1].memref,
                             ins.ins[1].offset), []).append(k)
                    else:
                        a = ins.ins[0]
                        lo_ldw.setdefault((a.memref, a.offset), []).append(k)
            attach = {}  # lo index -> list of hi indices to emit after it
            moved = set()
            for k, ins in enumerate(pe):
                if l1_half(ins) != 1 or not movable(ins):
                    continue
                if isinstance(ins, mybir.InstMatmult):
                    a, w = ins.ins[0], ins.ins[1]
                    key = (a.memref, a.offset - 64 * B, w.memref,
                           w.offset - 64 * H)
                    cands = lo_mm.get(key)
                else:
                    a = ins.ins[0]
                    cands = lo_ldw.get((a.memref, a.offset - 64 * H))
                if not cands:
                    continue
                part = min(cands, key=lambda c: abs(c - k))
                cands.remove(part)
                attach.setdefault(part, []).append(k)
                moved.add(k)

            if not moved:
                continue

            # rebuild the PE order with hi instrs attached to their partners
            new_pe = []
            for k, ins in enumerate(pe):
                if k in moved:
                    continue
                new_pe.append(ins)
                for hk in attach.get(k, ()):
                    new_pe.append(pe[hk])
            assert len(new_pe) == len(pe)

            # renumber waits on PE-exclusive semaphores: preserve the
            # trigger instruction of every wait value
            for sem in safe_sems:
                cum_old = []  # (inst id, cum) in old order
                cum = 0
                trig = {}
                for ins in pe:
                    si = ins.sync_info
                    inc = 0
                    if si is not None:
                        for u in (si.on_update or []):
                            if u.id == sem:
                                inc += u.update_value or 0
                    if inc:
                        lo_v = cum + 1
                        cum += inc
                        for v in range(lo_v, cum + 1):
                            trig[v] = id(ins)
                cum_new = {}
                cum = 0
                for ins in new_pe:
                    si = ins.sync_info
                    inc = 0
                    if si is not None:
                        for u in (si.on_update or []):
                            if u.id == sem:
                                inc += u.update_value or 0
                    if inc:
                        cum += inc
                        cum_new[id(ins)] = cum
                for ins in insts:
                    si = ins.sync_info
                    if si is None or not si.on_wait:
                        continue
                    changed = False
                    nw = []
                    for w in si.on_wait:
                        if (w.id == sem and w.wait_mode == "sem-ge-imm"
                                and w.wait_value and w.wait_value in trig):
                            nv = cum_new[trig[w.wait_value]]
                            if nv != w.wait_value:
                                changed = True
                            nw.append(mybir.SyncWait(
                                sync_type=w.sync_type, id=w.id,
                                ant_name=w.ant_name, wait_mode=w.wait_mode,
                                wait_value=nv,
                            ))
                        else:
                            nw.append(w)
                    if changed:
                        ins.sync_info = mybir.SyncInfo(
                            on_wait=nw, on_update=list(si.on_update or [])
                        )

            # write the reordered PE stream back into its original slots
            for i, ins in zip(pe_pos, new_pe):
                insts[i] = ins
            blk.instructions = insts
    return nc


def _legalize_waits(nc):
    """This walrus build encodes at most ONE semaphore wait per instruction;
    spill extras onto EventSemaphore instructions inserted just before, in
    the same engine's FIFO stream — semantically identical."""
    for fn in nc.m.functions:
        for blk in fn.blocks:
            new = []
            for inst in blk.instructions:
                si = inst.sync_info
                if si is not None and si.on_wait is not None and len(si.on_wait) > 1:
                    waits = list(si.on_wait)
                    for idx, w in enumerate(waits[:-1]):
                        new.append(mybir.InstEventSemaphore(
                            name=f"{inst.name}-spill{idx}",
                            engine=inst.engine,
                            sync_info=mybir.SyncInfo(on_wait=[w], on_update=[]),
                        ))
                    inst.sync_info = mybir.SyncInfo(
                        on_wait=[waits[-1]], on_update=list(si.on_update)
                    )
                new.append(inst)
            blk.instructions = new
    return nc


def build_graph():
    nc = bass.Bass()
    SIG_d = nc.declare_dram_parameter("SIG", [128, B], BF, False)
    # slot-0 prefix patterns: col P < 2^c holds sigma of pattern P on rows
    # k < c (step 0 of core c depends only on these <=128 distinct prefixes)
    SIGC_d = nc.declare_dram_parameter("SIGC", [128, 128], BF, False)
    V2_d = nc.declare_dram_parameter("V2", [NPAIR, 128, H], BF, False)
    W2_d = nc.declare_dram_parameter("W2", [NSTEP, 128, NM * H], FP8, False)
    CF_d = nc.declare_dram_parameter("CF", [128, NCF], F32, False)
    # DR-packed layer-3 weights: [p, (j*2+k2)*2+v, s, m] =
    # w3d[step j, (2k2+s)*128+p] on the m == 8v+j diagonal (32-row outputs,
    # rows 16-31 zero; DoubleRow is only legal at column position 0, so the
    # four batch chunks land on (bank b//2, rows 8*(b%2)+j))
    W3D_d = nc.declare_dram_parameter(
        "W3D", [128, NSTEP * 2 * 2 * 2 * 32], FP8, False)
    OUT_d = nc.declare_dram_parameter("out", [128, 2 * BCH], F32, True)

    add = mybir.AluOpType.add
    amax = mybir.AluOpType.max
    mult = mybir.AluOpType.mult
    Relu = mybir.ActivationFunctionType.Relu
    Exp = mybir.ActivationFunctionType.Exp
    Ln = mybir.ActivationFunctionType.Ln
    DR = mybir.MatmulPerfMode.DoubleRow

    with tile.TileContext(nc) as tc:
        with (
            tc.tile_pool(name="const", bufs=1) as const,
            tc.tile_pool(name="v2p", bufs=3) as v2p,
            tc.tile_pool(name="w2p", bufs=4) as w2p,
            tc.tile_pool(name="h1p", bufs=8) as h1p,
            tc.tile_pool(name="h2p", bufs=9) as h2p,
            tc.tile_pool(name="tailp", bufs=4) as tailp,
            tc.tile_pool(name="pq", bufs=3, space="PSUM") as pq,
            tc.tile_pool(name="dp", bufs=1, space="PSUM") as dp,
        ):
            # ---- warmup operands (memset'd; the PE warmup then starts as
            # soon as the GpSimd finishes its preamble, covering the
            # input-DMA wait)
            wz = const.tile([128, BCH + 128], BF)
            nc.vector.memset(wz[:], 0.0)
            wact = const.tile([128, 1], F32)
            nc.vector.memset(wact[:], 0.0)
            nc.scalar.activation(wact[:], wact[:], Exp)
            nc.scalar.activation(wact[:], wact[:], Ln)
            nc.scalar.activation(wact[:], wact[:], Relu)

            # ---- startup DMAs, first-use-ordered and spread across idle
            # engine queues (each DMA_DIRECT2D issue costs ~600ns serial on
            # its queue).  The first layer-1 slot needs V2[0] cols 0:128 and
            # SIG cols 0:1024; the big -sigma block of CF is tail-only and
            # goes last.
            v2_first = v2p.tile([128, H], BF, tag="v2")
            nc.sync.dma_start(out=v2_first[:, 0:256], in_=V2_d[0][:, 0:256])
            SIG_sb = const.tile([128, B], BF)
            nc.gpsimd.dma_start(out=SIG_sb[:, 0:BCH], in_=SIG_d[:, 0:BCH])
            SIGC_sb = const.tile([128, 128], BF)
            nc.scalar.dma_start(out=SIGC_sb[:], in_=SIGC_d[:])
            CF_sb = const.tile([128, NCF], F32)
            nc.scalar.dma_start(out=CF_sb[:, 0:C_NS], in_=CF_d[:, 0:C_NS])
            nc.sync.dma_start(out=v2_first[:, 256:H], in_=V2_d[0][:, 256:H])
            nc.sync.dma_start(
                out=SIG_sb[:, BCH:2 * BCH], in_=SIG_d[:, BCH:2 * BCH])
            nc.gpsimd.dma_start(
                out=SIG_sb[:, 2 * BCH:3 * BCH], in_=SIG_d[:, 2 * BCH:3 * BCH])
            nc.scalar.dma_start(
                out=SIG_sb[:, 3 * BCH:B], in_=SIG_d[:, 3 * BCH:B])
            w2_first = w2p.tile([128, NM, H], FP8, tag="w2")
            nc.sync.dma_start(
                out=w2_first[:],
                in_=W2_d[0].rearrange("p (k h) -> p k h", k=NM),
            )
            w2_second = w2p.tile([128, NM, H], FP8, tag="w2", name="w2_1")
            nc.gpsimd.dma_start(
                out=w2_second[:],
                in_=W2_d[1].rearrange("p (k h) -> p k h", k=NM),
            )
            W3D_sb = const.tile([128, NSTEP * 2 * 2, 2, 32], FP8)
            nc.gpsimd.dma_start(
                out=W3D_sb[:],
                in_=W3D_d.rearrange("p (j s m) -> p j s m", s=2, m=32),
            )
            nc.scalar.dma_start(
                out=CF_sb[:, C_NS:NCF], in_=CF_d[:, C_NS:NCF])

            # persistent d accumulators: bank t holds batch chunks 2t (rows
            # j) and 2t+1 (rows 8+j) via the two weight diagonals
            DB = [dp.tile([128, BCH], F32, name=f"D{t}") for t in range(2)]
            nc.vector.memset(DB[0][:], 0.0)
            nc.vector.memset(DB[1][:], 0.0)

            # PE HAM warmup fed from memset tiles (no DMA dependency, so
            # these start right after the preamble and warm the clock gate
            # during the input-DMA wait).  Targets the D accumulator bank
            # (writes zeros; layer 3 later re-opens the group with
            # start=True).  Read it afterwards so the matmuls aren't dead
            # code.
            for _ in range(6):
                nc.tensor.matmul(
                    DB[0][:, 0:BCH], wz[:, BCH:BCH + 128], wz[:, 0:BCH],
                    start=True, stop=True, skip_group_check=True,
                )
            nc.vector.tensor_copy(wact[:], DB[0][:, 0:1])

            # ---------- emit helpers ----------
            epi_ct = [0]

            def epilogue(dst, src, bias_col, eng=None):
                """relu(src + bias) -> dst, alternating ACT/DVE with a 7:6
                bias toward the faster ACT (1.2 vs 0.96 GHz; GPSIMD cannot
                read PSUM).  eng=0/1 forces ACT/DVE."""
                if eng is None:
                    eng = (epi_ct[0] % 13) % 2
                    epi_ct[0] += 1
                if eng == 0:
                    nc.scalar.activation(dst, src, Relu, bias=bias_col)
                else:
                    nc.vector.tensor_scalar(
                        dst, src, bias_col, 0.0, op0=add, op1=amax,
                    )

            def alloc_h1(j):
                # kk in {0,1}: [p, r, q] = h1'[(2kk+r)*128 + p, q]  (fp8)
                return [
                    h1p.tile([128, 2, B], FP8, tag="h1", name=f"h1_{j}_{kk}")
                    for kk in range(2)
                ]

            def emit_l1_slot(p, m, bp, v2t, h1lo, h1hi):
                """Concurrent row-tiled layer-1 for steps (2p, 2p+1):
                tile (0,0) computes step 2p's h-chunk m, tile (64,0) step
                2p+1's, over batch pair bp.  The interleaved (b0 A, b0 B,
                b1 A, b1 B) order lets the two array halves stream
                concurrently while Ldweights elide across b."""
                ps = [
                    pq.tile([128, 2 * BCH], F32, tag="ps",
                            name=f"q{half}_{p}_{m}_{bp}")
                    for half in range(2)
                ]
                for bsub in range(2):
                    b = 2 * bp + bsub
                    for half, rows in enumerate(((0, 64), (64, 128))):
                        nc.tensor.matmul(
                            ps[half][:, bsub * BCH:(bsub + 1) * BCH],
                            v2t[rows[0]:rows[1], m * 128:(m + 1) * 128],
                            SIG_sb[rows[0]:rows[1], b * BCH:(b + 1) * BCH],
                            start=True, stop=True,
                        )
                for half, h1t in enumerate((h1lo, h1hi)):
                    j = 2 * p + half
                    bias = CF_sb[:, C_C1 + j * NM + m: C_C1 + j * NM + m + 1]
                    dst = h1t[m // 2][:, m % 2, bp * 2 * BCH:(bp + 1) * 2 * BCH]
                    epilogue(dst, ps[half][:], bias)

            def emit_l2_block(j, m2, bp, w2, h1t, h2):
                """Layer-2 out-chunk m2, batch pair bp (fp8 DoubleRow,
                K=256/mm): h2' = relu(z' + 64 b2), z' accumulated over kk.
                kk-outer across the two batch chunks so consecutive matmuls
                share the stationary operand (Ldweights elision).  h2' is
                written fp8 into slot m2%2 of the pair tile m2//2 (the
                DoubleRow moving layout for layer 3)."""
                k2, slot = m2 // 2, m2 % 2
                h2m = h2.get(k2)
                if h2m is None:
                    h2m = h2p.tile([128, 2, B], FP8, tag="h2",
                                   name=f"h2_{j}_{k2}")
                    h2[k2] = h2m
                ps = pq.tile(
                    [128, 2 * BCH], F32, tag="ps", name=f"psh_{j}_{m2}_{bp}"
                )
                for kk in range(2):
                    for bsub in range(2):
                        b = 2 * bp + bsub
                        nc.tensor.matmul(
                            ps[:, bsub * BCH:(bsub + 1) * BCH],
                            w2[:, 2 * kk:2 * kk + 2, m2 * 128:(m2 + 1) * 128],
                            h1t[kk][:, :, b * BCH:(b + 1) * BCH],
                            start=(kk == 0), stop=(kk == 1),
                            perf_mode=DR,
                        )
                bias = CF_sb[:, C_B2 + j * NM + m2: C_B2 + j * NM + m2 + 1]
                epilogue(h2m[:, slot, bp * 2 * BCH:(bp + 1) * 2 * BCH],
                         ps[:], bias)

            def emit_l3(j, k2, h2m, bs=(0, 2, 1, 3)):
                """DB[b//2][8*(b%2) + j, :] += w3d_j^T @ h2' (fp8 DoubleRow,
                K=256/mm: k-chunks 2k2, 2k2+1).  32-row outputs at column
                position 0 (the only legal DoubleRow position); b-order
                0,2,1,3 shares each diagonal's Ldweights across the banks.
                All matmuls accumulate (start=False): the warmup/memset
                zeroed both banks, and overlapping diagonal writes must not
                clear each other's has_written bits."""
                if j == 0:
                    # cheap slot: one 128-col pattern chunk into bank 0,
                    # rows 0 AND 8 via the dual-diagonal weights
                    nc.tensor.matmul(
                        DB[0][0:32, 0:128],
                        W3D_sb[:, k2 * 2, :, :],
                        h2m[:, :, 0:128],
                        start=False, stop=False,
                        skip_group_check=True,
                        perf_mode=DR,
                    )
                    return
                for b in bs:
                    lhsT = W3D_sb[:, (j * 2 + k2) * 2 + (b % 2), :, :]
                    nc.tensor.matmul(
                        DB[b // 2][0:32, :],
                        lhsT,
                        h2m[:, :, b * BCH:(b + 1) * BCH],
                        start=False,
                        stop=(j == NSTEP - 1 and k2 == 1),
                        skip_group_check=True,
                        perf_mode=DR,
                    )

            def emit_l1_cheap(v2t, h1c):
                """Slot-0 layer 1 over the <=128 prefix patterns: 4 tiny
                128-col matmuls into one PSUM tile + 4 small epilogues."""
                ps = pq.tile([128, 2 * BCH], F32, tag="ps", name="psc1")
                for m in range(NM):
                    nc.tensor.matmul(
                        ps[:, m * 128:(m + 1) * 128],
                        v2t[0:64, m * 128:(m + 1) * 128],
                        SIGC_sb[0:64, 0:128],
                        start=True, stop=True,
                    )
                for m in range(NM):
                    bias = CF_sb[:, C_C1 + m: C_C1 + m + 1]
                    epilogue(h1c[m // 2][:, m % 2, 0:128],
                             ps[:, m * 128:(m + 1) * 128], bias)

            def emit_l1_hi(m, bp, v2t, h1t):
                """Full-B layer 1 for the hi half (step 1) of pair 0 only."""
                ps = pq.tile([128, 2 * BCH], F32, tag="ps",
                             name=f"qh_{m}_{bp}")
                for bsub in range(2):
                    b = 2 * bp + bsub
                    nc.tensor.matmul(
                        ps[:, bsub * BCH:(bsub + 1) * BCH],
                        v2t[64:128, m * 128:(m + 1) * 128],
                        SIG_sb[64:128, b * BCH:(b + 1) * BCH],
                        start=True, stop=True,
                    )
                bias = CF_sb[:, C_C1 + NM + m: C_C1 + NM + m + 1]
                epilogue(h1t[m // 2][:, m % 2, bp * 2 * BCH:(bp + 1) * 2 * BCH],
                         ps[:], bias)

            def emit_l2_cheap(m2, w2, h1c, h2, psd):
                """Slot-0 layer 2 over the pattern chunk (fp8 DoubleRow):
                all 4 m2 outputs share one PSUM tile, 128 cols each."""
                k2, slot = m2 // 2, m2 % 2
                h2m = h2.get(k2)
                if h2m is None:
                    h2m = h2p.tile([128, 2, 128], FP8, tag="h2c",
                                   name=f"h2c_{k2}")
                    h2[k2] = h2m
                ps = psd.get(0)
                if ps is None:
                    ps = pq.tile([128, 2 * BCH], F32, tag="ps", name="psc2")
                    psd[0] = ps
                c0 = m2 * 128
                for kk in range(2):
                    nc.tensor.matmul(
                        ps[:, c0:c0 + 128],
                        w2[:, 2 * kk:2 * kk + 2, m2 * 128:(m2 + 1) * 128],
                        h1c[kk][:, :, 0:128],
                        start=(kk == 0), stop=(kk == 1),
                        perf_mode=DR,
                    )
                bias = CF_sb[:, C_B2 + m2: C_B2 + m2 + 1]
                epilogue(h2m[:, slot, 0:128], ps[:, c0:c0 + 128], bias)

            # ---------- main pipeline ----------
            # prologue: slot-0 (pattern) layer 1 first, then the hi half
            # (step 1) at full B; bp-outer so later batch chunks of SIG are
            # needed later
            h1c_cur = [
                h1p.tile([128, 2, 128], FP8, tag="h1c", name=f"h1c_{kk}")
                for kk in range(2)
            ]
            h1_cur = [h1c_cur, alloc_h1(1)]
            emit_l1_cheap(v2_first, h1c_cur)
            for bp in range(NB // 2):
                for m in range(NM):
                    emit_l1_hi(m, bp, v2_first, h1_cur[1])

            # filler matmuls so the PE isn't idle while the first layer-1
            # epilogues drain through ACT/DVE
            for _ in range(3):
                nc.tensor.matmul(
                    DB[0][:, 0:BCH], wz[:, BCH:BCH + 128], wz[:, 0:BCH],
                    start=True, stop=True, skip_group_check=True,
                )
            nc.vector.tensor_copy(wact[:], DB[0][:, 0:1])

            w2t = [w2_first, w2_second]
            pend_l3 = []  # FIFO of deferred (j, k, h2) chunks, lag 2 blocks
            for p in range(NPAIR):
                if p + 1 < NPAIR:
                    v2n = v2p.tile([128, H], BF, tag="v2", name=f"v2_{p+1}")
                    nc.sync.dma_start(out=v2n[:], in_=V2_d[p + 1])
                    w2n = [None, None]
                    for t in range(2):
                        w2n[t] = w2p.tile(
                            [128, NM, H], FP8, tag="w2", name=f"w2_{2*p+2+t}"
                        )
                        nc.sync.dma_start(
                            out=w2n[t][:],
                            in_=W2_d[2 * p + 2 + t].rearrange(
                                "p (k h) -> p k h", k=NM
                            ),
                        )
                    h1_next = [alloc_h1(2 * p + 2), alloc_h1(2 * p + 3)]
                else:
                    v2n = w2n = h1_next = None

                l1q = [
                    (m, bp) for m in range(NM) for bp in range(NB // 2)
                ] if h1_next is not None else []
                l1i = [0]

                def maybe_l1():
                    if l1i[0] < len(l1q):
                        m, bp = l1q[l1i[0]]
                        l1i[0] += 1
                        emit_l1_slot(p + 1, m, bp, v2n, h1_next[0], h1_next[1])

                for half in range(2):
                    j = 2 * p + half
                    h2 = {}
                    psd = {}
                    for m2 in range(NM):
                        if j == 0:
                            emit_l2_cheap(m2, w2t[half], h1_cur[half], h2, psd)
                            if m2 % 2 == 1:
                                pend_l3.append((j, m2 // 2, h2[m2 // 2]))
                                if len(pend_l3) > 2:
                                    emit_l3(*pend_l3.pop(0))
                            maybe_l1()
                        else:
                            for bp in range(NB // 2):
                                emit_l2_block(j, m2, bp, w2t[half],
                                              h1_cur[half], h2)
                                if bp == 1:
                                    # lag the h2 consumers a full step
                                    # behind their epilogues; the alloc-free
                                    # layer-3 run buys the ACT/DVE queues
                                    # drain time right before the layer-1
                                    # slot's PSUM burst
                                    if m2 % 2 == 1:
                                        pend_l3.append(
                                            (j, m2 // 2, h2[m2 // 2]))
                                        if len(pend_l3) > 2:
                                            emit_l3(*pend_l3.pop(0))
                                    maybe_l1()

                h1_cur = h1_next
                w2t = w2n
            # ---- tail: -logp = ln(exp(u) + 1), u = (d + b3d) * (-sigma);
            # per-bank column-half chains on DVE (stt) and ACT (exp, then
            # ln with bias=1 — the +1 folds into the activation bias).
            # Bank 0's last layer-3 chunks flush first so its tail overlaps
            # bank 1's matmuls.
            def emit_tail(t):
                for ch in range(2):
                    k = 2 * t + ch
                    sl = slice(ch * (BCH // 2), (ch + 1) * (BCH // 2))
                    osl = slice(t * BCH + ch * (BCH // 2),
                                t * BCH + (ch + 1) * (BCH // 2))
                    nsl = slice(C_NS + t * BCH + ch * (BCH // 2),
                                C_NS + t * BCH + (ch + 1) * (BCH // 2))
                    tt = tailp.tile([128, BCH // 2], F32, tag="tt",
                                    name=f"tt{k}")
                    nc.vector.scalar_tensor_tensor(
                        tt[:], DB[t][:, sl], CF_sb[:, C_B3:C_B3 + 1],
                        CF_sb[:, nsl], op0=add, op1=mult,
                    )
                    ex = tailp.tile([128, BCH // 2], F32, tag="ex",
                                    name=f"ex{k}")
                    nc.scalar.activation(ex[:], tt[:], Exp)
                    lp = tailp.tile([128, BCH // 2], F32, tag="lp",
                                    name=f"lp{k}")
                    nc.scalar.activation(lp[:], ex[:], Ln, bias=1.0)
                    # alternate issue queues so the 4 out-DMA issues overlap
                    eng = nc.sync if k % 2 == 0 else nc.gpsimd
                    eng.dma_start(out=OUT_d[:, osl], in_=lp[:])

            for e in pend_l3:
                emit_l3(*e, bs=(0, 1))
            emit_tail(0)
            for e in pend_l3:
                emit_l3(*e, bs=(2, 3))
            emit_tail(1)

    nc = _drop_pe_self_waits(nc)
    if LOCKSTEP:
        nc = _pair_l1_lockstep(nc)
    return _legalize_waits(_thin_sem_incs(_elide_redundant_ldweights(nc)))


_NC_CACHE = None


def _get_graph():
    global _NC_CACHE
    if _NC_CACHE is None:
        _NC_CACHE = build_graph()
    return _NC_CACHE


def _prep_inputs(samples, W1, b1, W2, b2, W3, b3):
    samples = np.asarray(samples, np.float32)
    W1 = np.asarray(W1, np.float32)
    b1 = np.asarray(b1, np.float32)
    W2 = np.asarray(W2, np.float32)
    b2 = np.asarray(b2, np.float32)
    W3 = np.asarray(W3, np.float32)
    b3 = np.asarray(b3, np.float32)

    # sigma-encoding: W1[i, 2j+s] = M[i,j] + (1-2s) D[i,j]
    W1p = W1.reshape(N, N, 2, H)
    Mn = 0.5 * (W1p[:, :, 0] + W1p[:, :, 1])          # (N, N, H)
    Df = 0.5 * (W1p[:, :, 0] - W1p[:, :, 1])          # (N, N, H)
    jmask = np.arange(N)[None, :] < np.arange(N)[:, None]  # [i, j]: j < i
    Dm = np.where(jmask[:, :, None], Df * SH1, 0.0)   # masked, x8
    Cb = (b1 + (Mn * jmask[:, :, None]).sum(axis=1)) * SH1  # (N, H) exact c'

    sig = (samples[:, :, 0] - samples[:, :, 1]).astype(np.float32)  # (N, B)
    # SIG rows 0-63 and 64-127 both hold sigma (row-tiled duplicate)
    SIG = np.concatenate([sig, sig], axis=0).astype(NPBF)  # (128, B)

    w3d = ((W3[:, :, 0] - W3[:, :, 1]) / SZ2).astype(np.float32)  # (N, H)
    b3d = (b3[:, 0] - b3[:, 1]).astype(np.float32)                # (N,)

    in_maps = []
    for c in range(NCORES):
        steps = c + NCORES * np.arange(NSTEP)
        # V2[pair] rows 0-63 = D'[step 2p], rows 64-127 = D'[step 2p+1]
        V2c = np.concatenate(
            [
                np.stack([Dm[steps[2 * p]], Dm[steps[2 * p + 1]]], axis=0)
                .reshape(128, H)[None]
                for p in range(NPAIR)
            ],
            axis=0,
        ).astype(NPBF)
        W2c = (
            (W2[steps] * SW2)
            .reshape(NSTEP, NM, 128, H)
            .transpose(0, 2, 1, 3)
            .reshape(NSTEP, 128, NM * H)
            .astype(NPF8)
        )
        CF = np.zeros((128, NCF), np.float32)
        CF[:, C_C1:C_C1 + NSTEP * NM] = (
            Cb[steps].reshape(NSTEP, NM, 128).transpose(2, 0, 1)
            .reshape(128, NSTEP * NM)
        )
        CF[:, C_B2:C_B2 + NSTEP * NM] = (
            (b2[steps] * SZ2).reshape(NSTEP, NM, 128).transpose(2, 0, 1)
            .reshape(128, NSTEP * NM)
        )
        # tail row layout: bank t holds batch chunk 2t at rows j and chunk
        # 2t+1 at rows 8+j; -sigma is 0 on unused rows.  Slot 0 (j=0) is
        # the pattern slot: bank 0 rows 0/8 cols 0:128 hold the sigma=+1/-1
        # branches of every pattern (-sigma = -1/+1 there).
        b3col = np.zeros(128, np.float32)
        b3col[0:NSTEP] = b3d[steps]
        b3col[NSTEP:2 * NSTEP] = b3d[steps]
        CF[:, C_B3] = b3col
        nsg = -sig[steps].reshape(NSTEP, NB, BCH)  # (j, b, q)
        nsig = np.zeros((2, 128, BCH), np.float32)
        for t in range(2):
            nsig[t, 0:NSTEP, :] = nsg[:, 2 * t, :]
            nsig[t, NSTEP:2 * NSTEP, :] = nsg[:, 2 * t + 1, :]
        nsig[:, 0, :] = 0.0
        nsig[:, NSTEP, :] = 0.0
        nsig[0, 0, 0:128] = -1.0
        nsig[0, NSTEP, 0:128] = 1.0
        CF[:, C_NS:C_NS + 2 * BCH] = nsig.transpose(1, 0, 2).reshape(128, 2 * BCH)
        # W3D[p, (j*2+k2)*2+v, s, m] = w3d[steps[j], (2k2+s)*128+p] if
        # m == 8v+j (DoubleRow-packed lhsT; 32-wide outputs).  Slot 0
        # (cheap pattern slot) uses only v=0 with BOTH diagonals m=0 and
        # m=8 (sigma = +1 / -1 branches of the tail).
        W3Dc = np.zeros((128, NSTEP, 2, 2, 2, 32), np.float32)
        for j in range(NSTEP):
            wj = w3d[steps[j]].reshape(2, 2, 128).transpose(2, 0, 1)
            if j == 0:
                W3Dc[:, 0, :, 0, :, 0] = wj
                W3Dc[:, 0, :, 0, :, 8] = wj
            else:
                for v in range(2):
                    W3Dc[:, j, :, v, :, 8 * v + j] = wj
        W3Dc = W3Dc.reshape(128, NSTEP * 2 * 2 * 2 * 32).astype(NPF8)

        # slot-0 prefix patterns: col P holds sigma of pattern P on rows
        # k < c (and duplicated base rows stay zero above row c)
        c0 = steps[0]
        SIGC = np.zeros((128, 128), np.float32)
        if c0 > 0:
            P = np.arange(1 << c0)
            for k in range(c0):
                SIGC[k, 0:1 << c0] = 1.0 - 2.0 * ((P >> k) & 1)
        SIGC = SIGC.astype(NPBF)

        in_maps.append({
            "SIG": SIG,
            "SIGC": SIGC,
            "V2": V2c,
            "W2": W2c,
            "CF": CF,
            "W3D": W3Dc,
        })
    return in_maps


def kernel(samples, W1, b1, W2, b2, W3, b3):
    global LAST_RESULT
    nc = _get_graph()
    in_maps = _prep_inputs(samples, W1, b1, W2, b2, W3, b3)
    res = run_bass_kernel_spmd(
        nc, in_maps, core_ids=list(range(NCORES)), trace=TRACE,
    )
    LAST_RESULT = res
    # out col-block t rows j / 8+j hold -logp for (step j, batch chunk
    # 2t / 2t+1); rows 0 and 8 of bank 0 hold the slot-0 pattern branches
    idx = (np.asarray(samples)[:, :, 1] > 0.5).astype(np.int64)  # (N, B)
    acc = np.zeros(B, np.float64)
    for c in range(NCORES):
        o = np.asarray(res.results[c]["out"], np.float64)  # [128, 1024]
        for t in range(2):
            blk = o[:, t * BCH:(t + 1) * BCH]
            acc[(2 * t) * BCH:(2 * t + 1) * BCH] += blk[1:NSTEP].sum(axis=0)
            acc[(2 * t + 1) * BCH:(2 * t + 2) * BCH] += (
                blk[NSTEP + 1:2 * NSTEP].sum(axis=0)
            )
        # slot-0 (global step c): gather the per-pattern branch values
        blk0 = o[:, 0:BCH]
        if c == 0:
            pi = np.zeros(B, np.int64)
        else:
            pi = (idx[0:c] << np.arange(c)[:, None]).sum(axis=0)
        rows = np.where(idx[c] == 0, 0, NSTEP)
        acc += blk0[rows, pi]
    return (-acc).astype(np.float32).reshape(1, B)



# revision 89
# speedup vs baseline: 1.1824x; 1.1824x over previous
"""Trainium2 Bass kernel: autoregressive wavefunction log-prob (N=64, B=2048, H=512).

Sharding: step axis N across 8 cores, round-robin (core c owns global steps
c, c+8, ..., c+56).  Per local step i:
    h1 = relu(c_i + D_i^T sigma)        sigma-encoding of the one-hot prefix
    h2 = relu(h1 @ W2_i + b2_i)
    d  = h2 @ (W3_i[:,0]-W3_i[:,1]) + (b3_i[0]-b3_i[1])
    -logp_i = softplus(-sigma_i * d)

sigma-encoding: W1[2j+s] = M_j + sigma*D_j with sigma = s0-s1 in {+1,-1},
so h1_i = c_i + D_i^T sigma with contraction K=64.  Steps 2p/2p+1 run as
row-tiled tiles (0,0)/(64,0); a post-scheduling pass (LOCKSTEP) re-pairs
their matmuls adjacently so the two tiles stream concurrently (~2 cols/
cycle).  Layer 2 runs fp8 DoubleRow (K=256/pass) at the 1-col/cycle
roofline.  Layer 3 runs fp8 DoubleRow over h2' (pairs of k-chunks packed
in the moving layout), 32-row outputs at column position 0 (the only legal
DoubleRow position): batch chunk b lands in PSUM bank b//2 at rows
8*(b%2)+j via per-chunk weight diagonals.

Slot 0 (global step c) depends only on the <=2^c <= 128 distinct
sigma-prefix patterns: layers 1-3 run over an enumerated 128-pattern chunk
(exact), the tail emits both sigma=+1/-1 branches per pattern (bank-0 rows
0/8), and the host gathers per-sample values -- ~14% of the full-B work.

Scale folding keeps casts exact: D,c x8 (h1' = 8*h1 fp8), W2 x8 fp8,
b2 x64, h2' = 64*h2 fp8 (max ~158 < 240), W3D /64.  Tail:
-logp = ln(exp(u) + 1) with the +1 folded into the Ln activation bias,
bank-0 chunks flushed first so its tail overlaps bank 1's matmuls.
"""

import numpy as np
import ml_dtypes

import concourse.bass as bass
import concourse.mybir as mybir
import concourse.tile as tile
from concourse.bass_utils import run_bass_kernel_spmd

N, B, H = 64, 2048, 512
NCORES = 8
NSTEP = N // NCORES          # 8 local steps per core
NPAIR = NSTEP // 2           # 4 row-tiled step pairs
BCH = 512                    # batch chunk (one fp32 PSUM bank)
NB = B // BCH                # 4
NM = H // 128                # 4 h-chunks

BF = mybir.dt.bfloat16
F32 = mybir.dt.float32
FP8 = mybir.dt.float8e4
NPBF = ml_dtypes.bfloat16
NPF8 = ml_dtypes.float8_e4m3

SH1 = 8.0      # h1 scale (folded into D, c)
SW2 = 8.0      # W2 fp8 scale (8 keeps h2' = 64*h2 under fp8e4 max 240)
SZ2 = SH1 * SW2  # 64; folded into b2 and out of W3D

# CF32 packed const columns
C_C1 = 0                       # [0, 32)   c' bias, col j*NM+m
C_B2 = 32                      # [32, 64)  64*b2, col j*NM+m
C_B3 = 64                      # [64, 65)  b3d (rows j and 8+j)
C_NS = 65                      # [65, 65+2*BCH) -sigma, bank-major: bank t
#   holds batch chunks 2t (rows 0-7) and 2t+1 (rows 8-15)
NCF = C_NS + 2 * BCH

TRACE = False
LAST_RESULT = None
LOCKSTEP = True    # pair row-tiled L1 matmuls in the PE stream


def _thin_sem_incs(nc):
    """Drop PE-semaphore increments whose cumulative value nobody waits on
    (each then_inc serializes ~26ns at the EVT_SEM register); renumber the
    surviving waits.  PE executes in order, so the v-th increment firing
    implies all prior PE work retired.  Only touches semaphores updated
    exclusively by PE instructions with update_value 1."""
    blocks = [blk for fn in nc.m.functions for blk in fn.blocks]
    upd = {}
    waited = {}
    ok = {}
    for blk in blocks:
        for inst in blk.instructions:
            si = inst.sync_info
            if si is None:
                continue
            for w in (si.on_wait or []):
                if w.wait_mode == "sem-ge-imm" and w.wait_value is not None:
                    waited.setdefault(w.id, set()).add(w.wait_value)
                else:
                    ok[w.id] = False
            for u in (si.on_update or []):
                lst = upd.setdefault(u.id, [])
                lst.append((inst, (lst[-1][1] if lst else 0) + (u.update_value or 0)))
                is_pe = (
                    inst.engine == mybir.EngineType.PE
                    and u.update_value == 1
                    and getattr(u, "update_mode", "sem-inc") == "sem-inc"
                )
                if not is_pe:
                    ok[u.id] = False
    for sem_id, updates in upd.items():
        if not ok.get(sem_id, True):
            continue
        keep_vals = sorted(v for v in waited.get(sem_id, set()) if v > 0)
        remap = {}
        new_cum = 0
        ki = 0
        for inst, cum in updates:
            if ki < len(keep_vals) and cum >= keep_vals[ki]:
                while ki < len(keep_vals) and keep_vals[ki] <= cum:
                    remap[keep_vals[ki]] = new_cum + 1
                    ki += 1
                new_cum += 1
            else:
                si = inst.sync_info
                nu = [u for u in (si.on_update or []) if u.id != sem_id]
                inst.sync_info = mybir.SyncInfo(
                    on_wait=list(si.on_wait or []), on_update=nu
                )
        for blk in blocks:
            for inst in blk.instructions:
                si = inst.sync_info
                if si is None or not si.on_wait:
                    continue
                changed = False
                nw = []
                for w in si.on_wait:
                    if w.id == sem_id and w.wait_value and w.wait_value > 0:
                        nw.append(mybir.SyncWait(
                            sync_type=w.sync_type, id=w.id,
                            ant_name=w.ant_name, wait_mode=w.wait_mode,
                            wait_value=remap[w.wait_value],
                        ))
                        changed = True
                    else:
                        nw.append(w)
                if changed:
                    inst.sync_info = mybir.SyncInfo(
                        on_wait=nw, on_update=list(si.on_update or [])
                    )
    return nc


def _elide_redundant_ldweights(nc):
    """Tile lowers every matmul into an Ldweights+Matmult pair.  Consecutive
    matmuls sharing the stationary operand in the SAME array tile don't need
    the repeated loads (each tile_position owns independent weight
    registers, untouched by Matmults or by loads into other tiles).  A
    dropped load's semaphore waits are kept as an EventSemaphore in the PE
    stream."""
    for fn in nc.m.functions:
        for blk in fn.blocks:
            new = []
            last_key = {}  # tile_position -> weights key
            def region(inst):
                pos = inst.tile_position or (0, 0)
                ts = inst.tile_size or (128, 128)
                return (pos[0], pos[0] + ts[0], pos[1], pos[1] + ts[1])

            for inst in blk.instructions:
                if isinstance(inst, mybir.InstMatmult):
                    if inst.is_transpose:
                        last_key = {}  # transpose streams data in as weights
                    new.append(inst)
                    continue
                if isinstance(inst, mybir.InstLdweights):
                    a = inst.ins[0]
                    r = region(inst)
                    pos = (r[0], r[2])
                    key = (
                        a.memref, a.offset, str(a.ap), str(a.dtype),
                        str(inst.perf_mode), str(inst.is_transpose), r,
                    )
                    if last_key.get(pos) == key:
                        si = inst.sync_info
                        if si is not None and (si.on_wait or si.on_update):
                            new.append(mybir.InstEventSemaphore(
                                name=f"{inst.name}-ldwelide",
                                engine=inst.engine,
                                sync_info=si,
                            ))
                        continue
                    # this load clobbers every overlapping array region
                    for pos2 in list(last_key):
                        k2 = last_key[pos2]
                        r2 = k2[6]
                        if r[0] < r2[1] and r2[0] < r[1] and r[2] < r2[3] \
                                and r2[2] < r[3]:
                            del last_key[pos2]
                    last_key[pos] = key
                    new.append(inst)
                    continue
                # other PE-stream instructions (EventSemaphore, Drain,
                # RegisterMove, branches) don't touch the weight registers
                new.append(inst)
            blk.instructions = new
    return nc


def _drop_pe_self_waits(nc):
    """Remove PE-instruction waits on semaphores incremented exclusively by
    PE instructions.  The PE executes and drains strictly in program order,
    so any PE->PE ordering a semaphore enforces already holds — but the
    wait itself blocks ISSUE, which serializes row/column-tiled matmuls
    that the array could run concurrently.  Cross-engine waits (ACT/DVE/DMA
    waiting on PE, or PE waiting on them) are untouched."""
    blocks = [blk for fn in nc.m.functions for blk in fn.blocks]
    pe_only = {}
    for blk in blocks:
        for inst in blk.instructions:
            si = inst.sync_info
            if si is None:
                continue
            for u in (si.on_update or []):
                if inst.engine != mybir.EngineType.PE:
                    pe_only[u.id] = False
                elif u.id not in pe_only:
                    pe_only[u.id] = True
    for blk in blocks:
        new = []
        for inst in blk.instructions:
            si = inst.sync_info
            if inst.engine == mybir.EngineType.PE and si is not None and si.on_wait:
                kept = [
                    w for w in si.on_wait
                    if not (w.wait_mode == "sem-ge-imm" and pe_only.get(w.id, False))
                ]
                if len(kept) != len(si.on_wait):
                    if (not kept and not (si.on_update or [])
                            and isinstance(inst, mybir.InstEventSemaphore)):
                        continue  # nothing left to do
                    inst.sync_info = mybir.SyncInfo(
                        on_wait=kept, on_update=list(si.on_update or [])
                    )
            new.append(inst)
        blk.instructions = new
    return nc


def _pair_l1_lockstep(nc):
    """Interleave the row-tiled layer-1 matmul pairs in the final PE stream.

    Two concurrent 64-row tiles stream their moving columns in lockstep off
    the shared 128-partition XBUS read (~2 cols/cycle aggregate), but only
    when the (64,0)-tile matmul issues immediately after its (0,0) partner.
    The Tile scheduler's cost model serializes tiles, so it often groups
    same-tile matmuls instead.  This pass moves each hi-tile Ldweights/
    Matmult to sit right behind its lo partner and renumbers wait values on
    PE-exclusive semaphores so every wait still triggers on the same
    instruction.  Must run AFTER _drop_pe_self_waits (no PE->PE waits may
    remain) and BEFORE _elide_redundant_ldweights/_thin_sem_incs."""
    PE = mybir.EngineType.PE

    def l1_half(inst):
        if not isinstance(inst, (mybir.InstMatmult, mybir.InstLdweights)):
            return None
        if getattr(inst, "is_transpose", False):
            return None
        if inst.tile_size != (64, 128) or inst.tile_position is None:
            return None
        return 0 if inst.tile_position[0] == 0 else 1

    for fn in nc.m.functions:
        for blk in fn.blocks:
            insts = blk.instructions
            pe_pos = [i for i, ins in enumerate(insts)
                      if ins.engine == PE]
            pe = [insts[i] for i in pe_pos]
            if not pe:
                continue

            # semaphores updated exclusively by PE instructions in this blk
            pe_only = {}
            for ins in insts:
                si = ins.sync_info
                if si is None:
                    continue
                for u in (si.on_update or []):
                    if ins.engine != PE:
                        pe_only[u.id] = False
                    elif u.id not in pe_only:
                        pe_only[u.id] = True
            safe_sems = {s for s, ok in pe_only.items() if ok}

            def movable(ins):
                si = ins.sync_info
                if si is None:
                    return True
                return all(u.id in safe_sems for u in (si.on_update or []))

            # match each hi instruction to its lo partner
            lo_mm, lo_ldw = {}, {}
            for k, ins in enumerate(pe):
                if l1_half(ins) == 0:
                    if isinstance(ins, mybir.InstMatmult):
                        a = ins.ins[0]
                        lo_mm.setdefault(
                            (a.memref, a.offset, ins.ins[1].memref,
                             ins.ins[1].offset), []).append(k)
                    else:
                        a = ins.ins[0]
                        lo_ldw.setdefault((a.memref, a.offset), []).append(k)
            attach = {}  # lo index -> list of hi indices to emit after it
            moved = set()
            for k, ins in enumerate(pe):
                if l1_half(ins) != 1 or not movable(ins):
                    continue
                if isinstance(ins, mybir.InstMatmult):
                    a, w = ins.ins[0], ins.ins[1]
                    key = (a.memref, a.offset - 64 * B, w.memref,
                           w.offset - 64 * H)
                    cands = lo_mm.get(key)
                else:
                    a = ins.ins[0]
                    cands = lo_ldw.get((a.memref, a.offset - 64 * H))
                if not cands:
                    continue
                part = min(cands, key=lambda c: abs(c - k))
                cands.remove(part)
                attach.setdefault(part, []).append(k)
                moved.add(k)

            if not moved:
                continue

            # rebuild the PE order with hi instrs attached to their partners
            new_pe = []
            for k, ins in enumerate(pe):
                if k in moved:
                    continue
                new_pe.append(ins)
                for hk in attach.get(k, ()):
                    new_pe.append(pe[hk])
            assert len(new_pe) == len(pe)

            # renumber waits on PE-exclusive semaphores: preserve the
            # trigger instruction of every wait value
            for sem in safe_sems:
                cum_old = []  # (inst id, cum) in old order
                cum = 0
                trig = {}
                for ins in pe:
                    si = ins.sync_info
                    inc = 0
                    if si is not None:
                        for u in (si.on_update or []):
                            if u.id == sem:
                                inc += u.update_value or 0
                    if inc:
                        lo_v = cum + 1
                        cum += inc
                        for v in range(lo_v, cum + 1):
                            trig[v] = id(ins)
                cum_new = {}
                cum = 0
                for ins in new_pe:
                    si = ins.sync_info
                    inc = 0
                    if si is not None:
                        for u in (si.on_update or []):
                            if u.id == sem:
                                inc += u.update_value or 0
                    if inc:
                        cum += inc
                        cum_new[id(ins)] = cum
                for ins in insts:
                    si = ins.sync_info
                    if si is None or not si.on_wait:
                        continue
                    changed = False
                    nw = []
                    for w in si.on_wait:
                        if (w.id == sem and w.wait_mode == "sem-ge-imm"
                                and w.wait_value and w.wait_value in trig):
                            nv = cum_new[trig[w.wait_value]]
                            if nv != w.wait_value:
                                changed = True
                            nw.append(mybir.SyncWait(
                                sync_type=w.sync_type, id=w.id,
                                ant_name=w.ant_name, wait_mode=w.wait_mode,
                                wait_value=nv,
                            ))
                        else:
                            nw.append(w)
                    if changed:
                        ins.sync_info = mybir.SyncInfo(
                            on_wait=nw, on_update=list(si.on_update or [])
                        )

            # write the reordered PE stream back into its original slots
            for i, ins in zip(pe_pos, new_pe):
                insts[i] = ins
            blk.instructions = insts
    return nc


def _legalize_waits(nc):
    """This walrus build encodes at most ONE semaphore wait per instruction;
    spill extras onto EventSemaphore instructions inserted just before, in
    the same engine's FIFO stream — semantically identical."""
    for fn in nc.m.functions:
        for blk in fn.blocks:
            new = []
            for inst in blk.instructions:
                si = inst.sync_info
                if si is not None and si.on_wait is not None and len(si.on_wait) > 1:
                    waits = list(si.on_wait)
                    for idx, w in enumerate(waits[:-1]):
                        new.append(mybir.InstEventSemaphore(
                            name=f"{inst.name}-spill{idx}",
                            engine=inst.engine,
                            sync_info=mybir.SyncInfo(on_wait=[w], on_update=[]),
                        ))
                    inst.sync_info = mybir.SyncInfo(
                        on_wait=[waits[-1]], on_update=list(si.on_update)
                    )
                new.append(inst)
            blk.instructions = new
    return nc


def build_graph():
    nc = bass.Bass()
    SIG_d = nc.declare_dram_parameter("SIG", [128, B], BF, False)
    # slot-0 prefix patterns: col P < 2^c holds sigma of pattern P on rows
    # k < c (step 0 of core c depends only on these <=128 distinct prefixes)
    SIGC_d = nc.declare_dram_parameter("SIGC", [128, 128], BF, False)
    V2_d = nc.declare_dram_parameter("V2", [NPAIR, 128, H], BF, False)
    W2_d = nc.declare_dram_parameter("W2", [NSTEP, 128, NM * H], FP8, False)
    CF_d = nc.declare_dram_parameter("CF", [128, NCF], F32, False)
    # DR-packed layer-3 weights: [p, (j*2+k2)*2+v, s, m] =
    # w3d[step j, (2k2+s)*128+p] on the m == 8v+j diagonal (32-row outputs,
    # rows 16-31 zero; DoubleRow is only legal at column position 0, so the
    # four batch chunks land on (bank b//2, rows 8*(b%2)+j))
    W3D_d = nc.declare_dram_parameter(
        "W3D", [128, NSTEP * 2 * 2 * 2 * 32], FP8, False)
    OUT_d = nc.declare_dram_parameter("out", [128, 2 * BCH], F32, True)

    add = mybir.AluOpType.add
    amax = mybir.AluOpType.max
    mult = mybir.AluOpType.mult
    Relu = mybir.ActivationFunctionType.Relu
    Exp = mybir.ActivationFunctionType.Exp
    Ln = mybir.ActivationFunctionType.Ln
    DR = mybir.MatmulPerfMode.DoubleRow

    with tile.TileContext(nc) as tc:
        with (
            tc.tile_pool(name="const", bufs=1) as const,
            tc.tile_pool(name="v2p", bufs=3) as v2p,
            tc.tile_pool(name="w2p", bufs=4) as w2p,
            tc.tile_pool(name="h1p", bufs=8) as h1p,
            tc.tile_pool(name="h2p", bufs=9) as h2p,
            tc.tile_pool(name="tailp", bufs=4) as tailp,
            tc.tile_pool(name="pq", bufs=3, space="PSUM") as pq,
            tc.tile_pool(name="dp", bufs=1, space="PSUM") as dp,
        ):
            # ---- warmup operands (memset'd; the PE warmup then starts as
            # soon as the GpSimd finishes its preamble, covering the
            # input-DMA wait)
            wz = const.tile([128, BCH + 128], BF)
            nc.vector.memset(wz[:], 0.0)
            wact = const.tile([128, 1], F32)
            nc.vector.memset(wact[:], 0.0)
            nc.scalar.activation(wact[:], wact[:], Exp)
            nc.scalar.activation(wact[:], wact[:], Ln)
            nc.scalar.activation(wact[:], wact[:], Relu)

            # ---- startup DMAs, first-use-ordered and spread across idle
            # engine queues (each DMA_DIRECT2D issue costs ~600ns serial on
            # its queue).  The first layer-1 slot needs V2[0] cols 0:128 and
            # SIG cols 0:1024; the big -sigma block of CF is tail-only and
            # goes last.
            v2_first = v2p.tile([128, H], BF, tag="v2")
            nc.sync.dma_start(out=v2_first[:, 0:256], in_=V2_d[0][:, 0:256])
            SIG_sb = const.tile([128, B], BF)
            nc.gpsimd.dma_start(out=SIG_sb[:, 0:BCH], in_=SIG_d[:, 0:BCH])
            SIGC_sb = const.tile([128, 128], BF)
            nc.scalar.dma_start(out=SIGC_sb[:], in_=SIGC_d[:])
            CF_sb = const.tile([128, NCF], F32)
            nc.scalar.dma_start(out=CF_sb[:, 0:C_NS], in_=CF_d[:, 0:C_NS])
            nc.sync.dma_start(out=v2_first[:, 256:H], in_=V2_d[0][:, 256:H])
            nc.sync.dma_start(
                out=SIG_sb[:, BCH:2 * BCH], in_=SIG_d[:, BCH:2 * BCH])
            nc.gpsimd.dma_start(
                out=SIG_sb[:, 2 * BCH:3 * BCH], in_=SIG_d[:, 2 * BCH:3 * BCH])
            nc.scalar.dma_start(
                out=SIG_sb[:, 3 * BCH:B], in_=SIG_d[:, 3 * BCH:B])
            w2_first = w2p.tile([128, NM, H], FP8, tag="w2")
            nc.sync.dma_start(
                out=w2_first[:],
                in_=W2_d[0].rearrange("p (k h) -> p k h", k=NM),
            )
            w2_second = w2p.tile([128, NM, H], FP8, tag="w2", name="w2_1")
            nc.gpsimd.dma_start(
                out=w2_second[:],
                in_=W2_d[1].rearrange("p (k h) -> p k h", k=NM),
            )
            W3D_sb = const.tile([128, NSTEP * 2 * 2, 2, 32], FP8)
            nc.gpsimd.dma_start(
                out=W3D_sb[:],
                in_=W3D_d.rearrange("p (j s m) -> p j s m", s=2, m=32),
            )
            nc.scalar.dma_start(
                out=CF_sb[:, C_NS:NCF], in_=CF_d[:, C_NS:NCF])

            # persistent d accumulators: bank t holds batch chunks 2t (rows
            # j) and 2t+1 (rows 8+j) via the two weight diagonals
            DB = [dp.tile([128, BCH], F32, name=f"D{t}") for t in range(2)]
            nc.vector.memset(DB[0][:], 0.0)
            nc.vector.memset(DB[1][:], 0.0)

            # PE HAM warmup fed from memset tiles (no DMA dependency, so
            # these start right after the preamble and warm the clock gate
            # during the input-DMA wait).  Targets the D accumulator bank
            # (writes zeros; layer 3 later re-opens the group with
            # start=True).  Read it afterwards so the matmuls aren't dead
            # code.
            for _ in range(6):
                nc.tensor.matmul(
                    DB[0][:, 0:BCH], wz[:, BCH:BCH + 128], wz[:, 0:BCH],
                    start=True, stop=True, skip_group_check=True,
                )
            nc.vector.tensor_copy(wact[:], DB[0][:, 0:1])

            # ---------- emit helpers ----------
            epi_ct = [0]

            def epilogue(dst, src, bias_col, eng=None):
                """relu(src + bias) -> dst, alternating ACT/DVE (GPSIMD
                cannot read PSUM).  eng=0/1 forces ACT/DVE."""
                if eng is None:
                    eng = (epi_ct[0] % 13) % 2
                    epi_ct[0] += 1
                if eng == 0:
                    nc.scalar.activation(dst, src, Relu, bias=bias_col)
                else:
                    nc.vector.tensor_scalar(
                        dst, src, bias_col, 0.0, op0=add, op1=amax,
                    )

            def alloc_h1(j):
                # kk in {0,1}: [p, r, q] = h1'[(2kk+r)*128 + p, q]  (fp8)
                return [
                    h1p.tile([128, 2, B], FP8, tag="h1", name=f"h1_{j}_{kk}")
                    for kk in range(2)
                ]

            def emit_l1_slot(p, m, bp, v2t, h1lo, h1hi):
                """Concurrent row-tiled layer-1 for steps (2p, 2p+1):
                tile (0,0) computes step 2p's h-chunk m, tile (64,0) step
                2p+1's, over batch pair bp.  The interleaved (b0 A, b0 B,
                b1 A, b1 B) order lets the two array halves stream
                concurrently while Ldweights elide across b."""
                ps = [
                    pq.tile([128, 2 * BCH], F32, tag="ps",
                            name=f"q{half}_{p}_{m}_{bp}")
                    for half in range(2)
                ]
                for bsub in range(2):
                    b = 2 * bp + bsub
                    for half, rows in enumerate(((0, 64), (64, 128))):
                        nc.tensor.matmul(
                            ps[half][:, bsub * BCH:(bsub + 1) * BCH],
                            v2t[rows[0]:rows[1], m * 128:(m + 1) * 128],
                            SIG_sb[rows[0]:rows[1], b * BCH:(b + 1) * BCH],
                            start=True, stop=True,
                        )
                for half, h1t in enumerate((h1lo, h1hi)):
                    j = 2 * p + half
                    bias = CF_sb[:, C_C1 + j * NM + m: C_C1 + j * NM + m + 1]
                    dst = h1t[m // 2][:, m % 2, bp * 2 * BCH:(bp + 1) * 2 * BCH]
                    epilogue(dst, ps[half][:], bias)

            def emit_l2_block(j, m2, bp, w2, h1t, h2):
                """Layer-2 out-chunk m2, batch pair bp (fp8 DoubleRow,
                K=256/mm): h2' = relu(z' + 64 b2), z' accumulated over kk.
                kk-outer across the two batch chunks so consecutive matmuls
                share the stationary operand (Ldweights elision).  h2' is
                written fp8 into slot m2%2 of the pair tile m2//2 (the
                DoubleRow moving layout for layer 3)."""
                k2, slot = m2 // 2, m2 % 2
                h2m = h2.get(k2)
                if h2m is None:
                    h2m = h2p.tile([128, 2, B], FP8, tag="h2",
                                   name=f"h2_{j}_{k2}")
                    h2[k2] = h2m
                ps = pq.tile(
                    [128, 2 * BCH], F32, tag="ps", name=f"psh_{j}_{m2}_{bp}"
                )
                for kk in range(2):
                    for bsub in range(2):
                        b = 2 * bp + bsub
                        nc.tensor.matmul(
                            ps[:, bsub * BCH:(bsub + 1) * BCH],
                            w2[:, 2 * kk:2 * kk + 2, m2 * 128:(m2 + 1) * 128],
                            h1t[kk][:, :, b * BCH:(b + 1) * BCH],
                            start=(kk == 0), stop=(kk == 1),
                            perf_mode=DR,
                        )
                bias = CF_sb[:, C_B2 + j * NM + m2: C_B2 + j * NM + m2 + 1]
                epilogue(h2m[:, slot, bp * 2 * BCH:(bp + 1) * 2 * BCH],
                         ps[:], bias)

            def emit_l3(j, k2, h2m, bs=(0, 2, 1, 3)):
                """DB[b//2][8*(b%2) + j, :] += w3d_j^T @ h2' (fp8 DoubleRow,
                K=256/mm: k-chunks 2k2, 2k2+1).  32-row outputs at column
                position 0 (the only legal DoubleRow position); b-order
                0,2,1,3 shares each diagonal's Ldweights across the banks.
                All matmuls accumulate (start=False): the warmup/memset
                zeroed both banks, and overlapping diagonal writes must not
                clear each other's has_written bits."""
                if j == 0:
                    # cheap slot: one 128-col pattern chunk into bank 0,
                    # rows 0 AND 8 via the dual-diagonal weights
                    nc.tensor.matmul(
                        DB[0][0:32, 0:128],
                        W3D_sb[:, k2 * 2, :, :],
                        h2m[:, :, 0:128],
                        start=False, stop=False,
                        skip_group_check=True,
                        perf_mode=DR,
                    )
                    return
                for b in bs:
                    lhsT = W3D_sb[:, (j * 2 + k2) * 2 + (b % 2), :, :]
                    nc.tensor.matmul(
                        DB[b // 2][0:32, :],
                        lhsT,
                        h2m[:, :, b * BCH:(b + 1) * BCH],
                        start=False,
                        stop=(j == NSTEP - 1 and k2 == 1),
                        skip_group_check=True,
                        perf_mode=DR,
                    )

            def emit_l1_cheap(v2t, h1c):
                """Slot-0 layer 1 over the <=128 prefix patterns: 4 tiny
                128-col matmuls into one PSUM tile + 4 small epilogues."""
                ps = pq.tile([128, 2 * BCH], F32, tag="ps", name="psc1")
                for m in range(NM):
                    nc.tensor.matmul(
                        ps[:, m * 128:(m + 1) * 128],
                        v2t[0:64, m * 128:(m + 1) * 128],
                        SIGC_sb[0:64, 0:128],
                        start=True, stop=True,
                    )
                for m in range(NM):
                    bias = CF_sb[:, C_C1 + m: C_C1 + m + 1]
                    epilogue(h1c[m // 2][:, m % 2, 0:128],
                             ps[:, m * 128:(m + 1) * 128], bias)

            def emit_l1_hi(m, bp, v2t, h1t):
                """Full-B layer 1 for the hi half (step 1) of pair 0 only."""
                ps = pq.tile([128, 2 * BCH], F32, tag="ps",
                             name=f"qh_{m}_{bp}")
                for bsub in range(2):
                    b = 2 * bp + bsub
                    nc.tensor.matmul(
                        ps[:, bsub * BCH:(bsub + 1) * BCH],
                        v2t[64:128, m * 128:(m + 1) * 128],
                        SIG_sb[64:128, b * BCH:(b + 1) * BCH],
                        start=True, stop=True,
                    )
                bias = CF_sb[:, C_C1 + NM + m: C_C1 + NM + m + 1]
                epilogue(h1t[m // 2][:, m % 2, bp * 2 * BCH:(bp + 1) * 2 * BCH],
                         ps[:], bias)

            def emit_l2_cheap(m2, w2, h1c, h2, psd):
                """Slot-0 layer 2 over the pattern chunk (fp8 DoubleRow):
                all 4 m2 outputs share one PSUM tile, 128 cols each."""
                k2, slot = m2 // 2, m2 % 2
                h2m = h2.get(k2)
                if h2m is None:
                    h2m = h2p.tile([128, 2, 128], FP8, tag="h2c",
                                   name=f"h2c_{k2}")
                    h2[k2] = h2m
                ps = psd.get(0)
                if ps is None:
                    ps = pq.tile([128, 2 * BCH], F32, tag="ps", name="psc2")
                    psd[0] = ps
                c0 = m2 * 128
                for kk in range(2):
                    nc.tensor.matmul(
                        ps[:, c0:c0 + 128],
                        w2[:, 2 * kk:2 * kk + 2, m2 * 128:(m2 + 1) * 128],
                        h1c[kk][:, :, 0:128],
                        start=(kk == 0), stop=(kk == 1),
                        perf_mode=DR,
                    )
                bias = CF_sb[:, C_B2 + m2: C_B2 + m2 + 1]
                epilogue(h2m[:, slot, 0:128], ps[:, c0:c0 + 128], bias)

            # ---------- main pipeline ----------
            # prologue: slot-0 (pattern) layer 1 first, then the hi half
            # (step 1) at full B; bp-outer so later batch chunks of SIG are
            # needed later
            h1c_cur = [
                h1p.tile([128, 2, 128], FP8, tag="h1c", name=f"h1c_{kk}")
                for kk in range(2)
            ]
            h1_cur = [h1c_cur, alloc_h1(1)]
            emit_l1_cheap(v2_first, h1c_cur)
            for bp in range(NB // 2):
                for m in range(NM):
                    emit_l1_hi(m, bp, v2_first, h1_cur[1])

            # filler matmuls so the PE isn't idle while the first layer-1
            # epilogues drain through ACT/DVE
            for _ in range(3):
                nc.tensor.matmul(
                    DB[0][:, 0:BCH], wz[:, BCH:BCH + 128], wz[:, 0:BCH],
                    start=True, stop=True, skip_group_check=True,
                )
            nc.vector.tensor_copy(wact[:], DB[0][:, 0:1])

            w2t = [w2_first, w2_second]
            pend_l3 = []  # FIFO of deferred (j, k, h2) chunks, lag 2 blocks
            for p in range(NPAIR):
                if p + 1 < NPAIR:
                    v2n = v2p.tile([128, H], BF, tag="v2", name=f"v2_{p+1}")
                    nc.sync.dma_start(out=v2n[:], in_=V2_d[p + 1])
                    w2n = [None, None]
                    for t in range(2):
                        w2n[t] = w2p.tile(
                            [128, NM, H], FP8, tag="w2", name=f"w2_{2*p+2+t}"
                        )
                        nc.sync.dma_start(
                            out=w2n[t][:],
                            in_=W2_d[2 * p + 2 + t].rearrange(
                                "p (k h) -> p k h", k=NM
                            ),
                        )
                    h1_next = [alloc_h1(2 * p + 2), alloc_h1(2 * p + 3)]
                else:
                    v2n = w2n = h1_next = None

                l1q = [
                    (m, bp) for m in range(NM) for bp in range(NB // 2)
                ] if h1_next is not None else []
                l1i = [0]

                def maybe_l1():
                    if l1i[0] < len(l1q):
                        m, bp = l1q[l1i[0]]
                        l1i[0] += 1
                        emit_l1_slot(p + 1, m, bp, v2n, h1_next[0], h1_next[1])

                for half in range(2):
                    j = 2 * p + half
                    h2 = {}
                    psd = {}
                    for m2 in range(NM):
                        if j == 0:
                            emit_l2_cheap(m2, w2t[half], h1_cur[half], h2, psd)
                            if m2 % 2 == 1:
                                pend_l3.append((j, m2 // 2, h2[m2 // 2]))
                                if len(pend_l3) > 2:
                                    emit_l3(*pend_l3.pop(0))
                            maybe_l1()
                        else:
                            for bp in range(NB // 2):
                                emit_l2_block(j, m2, bp, w2t[half],
                                              h1_cur[half], h2)
                                if bp == 1:
                                    # lag the h2 consumers a full step
                                    # behind their epilogues; the alloc-free
                                    # layer-3 run buys the ACT/DVE queues
                                    # drain time right before the layer-1
                                    # slot's PSUM burst
                                    if m2 % 2 == 1:
                                        pend_l3.append(
                                            (j, m2 // 2, h2[m2 // 2]))
                                        if len(pend_l3) > 2:
                                            emit_l3(*pend_l3.pop(0))
                                    maybe_l1()

                h1_cur = h1_next
                w2t = w2n
            # ---- tail: -logp = ln(exp(u) + 1), u = (d + b3d) * (-sigma);
            # per-bank column-half chains on DVE (stt) and ACT (exp, then
            # ln with bias=1 — the +1 folds into the activation bias).
            # Bank 0's last layer-3 chunks flush first so its tail overlaps
            # bank 1's matmuls.
            def emit_tail(t):
                for ch in range(2):
                    k = 2 * t + ch
                    sl = slice(ch * (BCH // 2), (ch + 1) * (BCH // 2))
                    osl = slice(t * BCH + ch * (BCH // 2),
                                t * BCH + (ch + 1) * (BCH // 2))
                    nsl = slice(C_NS + t * BCH + ch * (BCH // 2),
                                C_NS + t * BCH + (ch + 1) * (BCH // 2))
                    tt = tailp.tile([128, BCH // 2], F32, tag="tt",
                                    name=f"tt{k}")
                    nc.vector.scalar_tensor_tensor(
                        tt[:], DB[t][:, sl], CF_sb[:, C_B3:C_B3 + 1],
                        CF_sb[:, nsl], op0=add, op1=mult,
                    )
                    ex = tailp.tile([128, BCH // 2], F32, tag="ex",
                                    name=f"ex{k}")
                    nc.scalar.activation(ex[:], tt[:], Exp)
                    lp = tailp.tile([128, BCH // 2], F32, tag="lp",
                                    name=f"lp{k}")
                    nc.scalar.activation(lp[:], ex[:], Ln, bias=1.0)
                    # alternate issue queues so the 4 out-DMA issues overlap
                    eng = nc.sync if k % 2 == 0 else nc.gpsimd
                    eng.dma_start(out=OUT_d[:, osl], in_=lp[:])

            for e in pend_l3:
                emit_l3(*e, bs=(0, 1))
            emit_tail(0)
            for e in pend_l3:
                emit_l3(*e, bs=(2, 3))
            emit_tail(1)

    nc = _drop_pe_self_waits(nc)
    if LOCKSTEP:
        nc = _pair_l1_lockstep(nc)
    return _legalize_waits(_thin_sem_incs(_elide_redundant_ldweights(nc)))


_NC_CACHE = None


def _get_graph():
    global _NC_CACHE
    if _NC_CACHE is None:
        _NC_CACHE = build_graph()
    return _NC_CACHE


def _prep_inputs(samples, W1, b1, W2, b2, W3, b3):
    samples = np.asarray(samples, np.float32)
    W1 = np.asarray(W1, np.float32)
    b1 = np.asarray(b1, np.float32)
    W2 = np.asarray(W2, np.float32)
    b2 = np.asarray(b2, np.float32)
    W3 = np.asarray(W3, np.float32)
    b3 = np.asarray(b3, np.float32)

    # sigma-encoding: W1[i, 2j+s] = M[i,j] + (1-2s) D[i,j]
    W1p = W1.reshape(N, N, 2, H)
    Mn = 0.5 * (W1p[:, :, 0] + W1p[:, :, 1])          # (N, N, H)
    Df = 0.5 * (W1p[:, :, 0] - W1p[:, :, 1])          # (N, N, H)
    jmask = np.arange(N)[None, :] < np.arange(N)[:, None]  # [i, j]: j < i
    Dm = np.where(jmask[:, :, None], Df * SH1, 0.0)   # masked, x8
    Cb = (b1 + (Mn * jmask[:, :, None]).sum(axis=1)) * SH1  # (N, H) exact c'

    sig = (samples[:, :, 0] - samples[:, :, 1]).astype(np.float32)  # (N, B)
    # SIG rows 0-63 and 64-127 both hold sigma (row-tiled duplicate)
    SIG = np.concatenate([sig, sig], axis=0).astype(NPBF)  # (128, B)

    w3d = ((W3[:, :, 0] - W3[:, :, 1]) / SZ2).astype(np.float32)  # (N, H)
    b3d = (b3[:, 0] - b3[:, 1]).astype(np.float32)                # (N,)

    in_maps = []
    for c in range(NCORES):
        steps = c + NCORES * np.arange(NSTEP)
        # V2[pair] rows 0-63 = D'[step 2p], rows 64-127 = D'[step 2p+1]
        V2c = np.concatenate(
            [
                np.stack([Dm[steps[2 * p]], Dm[steps[2 * p + 1]]], axis=0)
                .reshape(128, H)[None]
                for p in range(NPAIR)
            ],
            axis=0,
        ).astype(NPBF)
        W2c = (
            (W2[steps] * SW2)
            .reshape(NSTEP, NM, 128, H)
            .transpose(0, 2, 1, 3)
            .reshape(NSTEP, 128, NM * H)
            .astype(NPF8)
        )
        CF = np.zeros((128, NCF), np.float32)
        CF[:, C_C1:C_C1 + NSTEP * NM] = (
            Cb[steps].reshape(NSTEP, NM, 128).transpose(2, 0, 1)
            .reshape(128, NSTEP * NM)
        )
        CF[:, C_B2:C_B2 + NSTEP * NM] = (
            (b2[steps] * SZ2).reshape(NSTEP, NM, 128).transpose(2, 0, 1)
            .reshape(128, NSTEP * NM)
        )
        # tail row layout: bank t holds batch chunk 2t at rows j and chunk
        # 2t+1 at rows 8+j; -sigma is 0 on unused rows.  Slot 0 (j=0) is
        # the pattern slot: bank 0 rows 0/8 cols 0:128 hold the sigma=+1/-1
        # branches of every pattern (-sigma = -1/+1 there).
        b3col = np.zeros(128, np.float32)
        b3col[0:NSTEP] = b3d[steps]
        b3col[NSTEP:2 * NSTEP] = b3d[steps]
        CF[:, C_B3] = b3col
        nsg = -sig[steps].reshape(NSTEP, NB, BCH)  # (j, b, q)
        nsig = np.zeros((2, 128, BCH), np.float32)
        for t in range(2):
            nsig[t, 0:NSTEP, :] = nsg[:, 2 * t, :]
            nsig[t, NSTEP:2 * NSTEP, :] = nsg[:, 2 * t + 1, :]
        nsig[:, 0, :] = 0.0
        nsig[:, NSTEP, :] = 0.0
        nsig[0, 0, 0:128] = -1.0
        nsig[0, NSTEP, 0:128] = 1.0
        CF[:, C_NS:C_NS + 2 * BCH] = nsig.transpose(1, 0, 2).reshape(128, 2 * BCH)
        # W3D[p, (j*2+k2)*2+v, s, m] = w3d[steps[j], (2k2+s)*128+p] if
        # m == 8v+j (DoubleRow-packed lhsT; 32-wide outputs).  Slot 0
        # (cheap pattern slot) uses only v=0 with BOTH diagonals m=0 and
        # m=8 (sigma = +1 / -1 branches of the tail).
        W3Dc = np.zeros((128, NSTEP, 2, 2, 2, 32), np.float32)
        for j in range(NSTEP):
            wj = w3d[steps[j]].reshape(2, 2, 128).transpose(2, 0, 1)
            if j == 0:
                W3Dc[:, 0, :, 0, :, 0] = wj
                W3Dc[:, 0, :, 0, :, 8] = wj
            else:
                for v in range(2):
                    W3Dc[:, j, :, v, :, 8 * v + j] = wj
        W3Dc = W3Dc.reshape(128, NSTEP * 2 * 2 * 2 * 32).astype(NPF8)

        # slot-0 prefix patterns: col P holds sigma of pattern P on rows
        # k < c (and duplicated base rows stay zero above row c)
        c0 = steps[0]
        SIGC = np.zeros((128, 128), np.float32)
        if c0 > 0:
            P = np.arange(1 << c0)
            for k in range(c0):
                SIGC[k, 0:1 << c0] = 1.0 - 2.0 * ((P >> k) & 1)
        SIGC = SIGC.astype(NPBF)

        in_maps.append({
            "SIG": SIG,
            "SIGC": SIGC,
            "V2": V2c,
            "W2": W2c,
            "CF": CF,
            "W3D": W3Dc,
        })
    return in_maps


def kernel(samples, W1, b1, W2, b2, W3, b3):
    global LAST_RESULT
    nc = _get_graph()
    in_maps = _prep_inputs(samples, W1, b1, W2, b2, W3, b3)
    res = run_bass_kernel_spmd(
        nc, in_maps, core_ids=list(range(NCORES)), trace=TRACE,
    )
    LAST_RESULT = res
    # out col-block t rows j / 8+j hold -logp for (step j, batch chunk
    # 2t / 2t+1); rows 0 and 8 of bank 0 hold the slot-0 pattern branches
    idx = (np.asarray(samples)[:, :, 1] > 0.5).astype(np.int64)  # (N, B)
    acc = np.zeros(B, np.float64)
    for c in range(NCORES):
        o = np.asarray(res.results[c]["out"], np.float64)  # [128, 1024]
        for t in range(2):
            blk = o[:, t * BCH:(t + 1) * BCH]
            acc[(2 * t) * BCH:(2 * t + 1) * BCH] += blk[1:NSTEP].sum(axis=0)
            acc[(2 * t + 1) * BCH:(2 * t + 2) * BCH] += (
                blk[NSTEP + 1:2 * NSTEP].sum(axis=0)
            )
        # slot-0 (global step c): gather the per-pattern branch values
        blk0 = o[:, 0:BCH]
        if c == 0:
            pi = np.zeros(B, np.int64)
        else:
            pi = (idx[0:c] << np.arange(c)[:, None]).sum(axis=0)
        rows = np.where(idx[c] == 0, 0, NSTEP)
        acc += blk0[rows, pi]
    return (-acc).astype(np.float32).reshape(1, B)

